# revision 1
# baseline (speedup 1.0000x reference)
"""Trainium2 Bass kernel for per-sample dynamic 3x3 conv (periodic padding).

y[b,o,h,w] = sum_{c,i,j} x[b,c,(h+i-1)%H,(w+j-1)%W] * wgt[b, c*9+i*3+j, o] + bias[b,o]

Shapes: x [16,64,128,128] f32, wgt [16,576,64] f32, bias [16,64] f32.

Sharding: data-parallel over batch, 2 samples per core on 8 cores.

Compute scheme: both per-core samples are packed into single 128x128
matmuls with block-diagonal stationary weights:
  lhsT[k,m] = W_s0[c,o] at (k=c, m=o), W_s1[c,o] at (k=64+c, m=64+o), else 0
  rhs[k,n]  = col-padded img_s0[c, pos] (k<64) / img_s1[c, pos] (k>=64)
so one matmul per 3x3 shift contracts C=64 for both samples at once
(full K=128, full M=128, N=512). Matmuls are float32r (1 cycle/row).

Data movement: the 16 DMA engines need 128-partition transfers with
large contiguous per-partition runs to reach ~400GB/s (partitions map
to engines mod 16; per-descriptor overhead ~300ns), so images load
contiguously into a raw SBUF tile with both samples merged into one
(b c)-partition DMA per row chunk, and stores go out 32 image rows at
a time the same way. The column-wrap-padded image [128, 128, 130] is
built on-chip by DVE/ACT/GPSIMD copies (DVE owns the rows the matmul
stream needs early); the row wrap is handled by splitting the affected
matmuls on the two boundary spatial tiles, with row H-1 loaded first.
"""

import numpy as np

KH = KW = 3
B, C, O, H, W = 16, 64, 64, 128, 128
N_CORES = 8
BPC = B // N_CORES  # samples per core
WP = W + 2  # 130: column-wrap padded row length
TILE_ROWS = 4  # output rows per PSUM tile -> N = 4*128 = 512
N_TILES = H // TILE_ROWS
LOAD_CHUNK = 32  # image rows per interior load DMA / pad-build chunk
OGROUP = 8  # spatial tiles per output store group (32 rows)

_CACHE = {}


def _patch_tile_drain():
    """This container's walrus rejects Drain instructions carrying more than
    one sem wait (setupSyncWait: Too many sync wait commands). Re-emit the
    TileContext exit drain's waits as individual wait_ge instructions."""
    import concourse.tile as tile
    from concourse.vector_clock import ScopedClock

    if getattr(tile.TileContext, "_drain_patch_applied", False):
        return

    def _drain_and_barrier(self, tick_clock, wait_clock):
        nc = self.nc
        nop = nc.sync.nop(nofuse=True)
        wait_clock.add_sem_waits(nop.ins, ScopedClock({None: tick_clock.global_clock}))
        waits = list(nop.ins.sync_info.on_wait)
        nop.ins.sync_info.on_wait.clear()
        assert self.sems is not None
        by_name = {}
        for h in self.sems.allocated().values():
            by_name[getattr(h, "name", None)] = h
        for w in waits:
            h = by_name.get(w.ant_name)
            assert h is not None, f"no sem handle for {w.ant_name}"
            nc.sync.wait_ge(h, w.wait_value)
        nc.sync.drain()
        nc.all_engine_barrier()
        popped = nc._tile_sem_poison_stack.pop()
        assert popped is self._sem_poison
        nc.clear_and_free_semaphores(list(self.sems.allocated().values()))

    tile.TileContext._drain_and_barrier = _drain_and_barrier
    tile.TileContext._drain_patch_applied = True


def _split_multi_waits(nc, max_waits=1):
    """Same walrus limitation, general form: any instruction carrying more
    than one sem wait fails setupSyncWait. Hoist excess waits onto dedicated
    single-wait NOPs on the same engine, placed just before the instruction."""
    import concourse.mybir as mybir

    for f in nc.m.functions:
        for blk in f.blocks:
            out = []
            changed = False
            for inst in blk.instructions:
                si = getattr(inst, "sync_info", None)
                waits = list(si.on_wait) if si is not None else []
                if len(waits) > max_waits:
                    changed = True
                    for w in waits[:-max_waits]:
                        out.append(
                            mybir.InstNoOp(
                                name=nc.get_next_instruction_name(),
                                engine=inst.engine,
                                sync_info=mybir.SyncInfo(on_wait=[w], on_update=[]),
                                bass_nofuse=True,
                            )
                        )
                    si.on_wait.clear()
                    for w in waits[-max_waits:]:
                        si.on_wait.append(w)
                out.append(inst)
            if changed:
                blk.instructions = out


def _build_module():
    import concourse.bass as bass
    import concourse.mybir as mybir
    import concourse.tile as tile

    _patch_tile_drain()

    f32 = mybir.dt.float32
    f32r = mybir.dt.float32r

    nc = bass.Bass()
    # input/weight feed FP32r matmuls; the BIR verifier requires every
    # producer in that dataflow to be float32r-typed, so declare the whole
    # chain float32r. float32r is byte-identical to float32 host-side.
    x_d = nc.dram_tensor("input", [BPC, C, H, W], f32r, kind="ExternalInput")
    # block-diag weights are pre-assembled host-side in _in_maps:
    # wbd[p, s, m] with wbd[c, s, o] = W_s0, wbd[64+c, s, 64+o] = W_s1, 0 else
    w_d = nc.dram_tensor(
        "wbd", [128, KH * KW, 128], f32r, kind="ExternalInput"
    )
    b_d = nc.dram_tensor("bias", [BPC, O], f32, kind="ExternalInput")
    y_d = nc.dram_tensor("out", [BPC, O, H, W], f32, kind="ExternalOutput")

    with tile.TileContext(nc) as tc:
        from contextlib import ExitStack

        ctx = ExitStack()
        with ctx:
            persist = ctx.enter_context(tc.tile_pool(name="persist", bufs=1))
            psum = ctx.enter_context(tc.tile_pool(name="psum", bufs=6, space="PSUM"))
            ostage = ctx.enter_context(tc.tile_pool(name="ostage", bufs=2))

            # --- first image rows ahead of everything: tile-0's padded
            # build can then overlap the weight load instead of trailing it
            raw = persist.tile([128, H, W], f32r)
            x_bc = x_d.rearrange("b c h w -> (b c) h w")
            for r0, nr in [(H - 1, 1), (0, 8)]:
                nc.sync.dma_start(
                    out=raw[:, r0 : r0 + nr, :], in_=x_bc[:, r0 : r0 + nr, :]
                )

            # --- weights (every matmul needs them; 590KB) ---
            wts = persist.tile([128, KH * KW, 128], f32r)
            nc.sync.dma_start(out=wts, in_=w_d[:, :, :])

            # --- bias + ACT table preload (Identity's act table costs 1.3us
            # on first use; trigger it during the load phase) ---
            bias_sb = persist.tile([128, 1], f32)
            nc.sync.dma_start(
                out=bias_sb,
                in_=b_d.rearrange("b o -> (b o)").rearrange("(p x) -> p x", x=1),
            )
            act_warm = persist.tile([128, 1], f32)
            nc.scalar.activation(
                out=act_warm, in_=bias_sb,
                func=mybir.ActivationFunctionType.Identity, bias=bias_sb,
            )

            # --- raw images, fully contiguous loads: [128 parts, 128*128].
            # Row H-1 loads first (tile 0 reads it through the periodic
            # wrap); then small-to-large chunks so tile-0 compute starts
            # within a few us while the bulk streams in behind it.
            # Each DMA must span all 128 partitions: partitions map to the 16
            # DMA engines mod-16, so a 64-partition DMA runs each engine at
            # half throughput (measured 13 vs 26.5 GB/s per engine).
            load_rows = [(8, 24), (32, 32), (64, H - 1 - 64)]
            for r0, nr in load_rows:
                nc.sync.dma_start(
                    out=raw[:, r0 : r0 + nr, :],
                    in_=x_bc[:, r0 : r0 + nr, :],
                )


            # --- column-wrap padded image [128, 128, 130], built on-chip.
            # img[c, r, 0] = x[c, r, 127]; img[c, r, 1:129] = x[c, r, :];
            # img[c, r, 129] = x[c, r, 0]. Row wrap is NOT padded (handled by
            # split matmuls on boundary tiles). Spread copies across engines.
            y_bo = y_d.rearrange("b o h w -> (b o) h w")
            img = persist.tile([128, H, WP], f32r)
            # (rows, engine): DVE is ~6x faster than GpSimd at these strided
            # copies, so DVE takes the early-needed rows; GpSimd only gets
            # rows not consumed until late in the matmul stream.
            # DVE handles every build the matmul stream needs early (it's the
            # fastest and now has no other work); the late rows go to the
            # otherwise-idle GpSimd; ACT gets one chunk before its merges.
            build_rows = [
                ((H - 1, 1), nc.vector),
                ((0, 8), nc.vector),
                ((8, 24), nc.scalar),
                ((32, 32), nc.vector),
                ((64, 32), nc.vector),
                ((96, H - 1 - 96), nc.gpsimd),
            ]

            def eng_copy(e, out, in_):
                if e is nc.scalar:
                    e.activation(
                        out=out, in_=in_, func=mybir.ActivationFunctionType.Copy
                    )
                else:
                    e.tensor_copy(out=out, in_=in_)

            for (r0, nr), e in build_rows:
                r1 = r0 + nr
                eng_copy(e, img[:, r0:r1, 1 : 1 + W], raw[:, r0:r1, :])
                eng_copy(e, img[:, r0:r1, 0], img[:, r0:r1, W])
                eng_copy(e, img[:, r0:r1, WP - 1], img[:, r0:r1, 1])

            # --- main loop: 32 spatial tiles of 4 output rows.
            # Shift row order [1, 0, 2] so the first matmul of each tile is
            # always a full-coverage N=512 one (start=True zeroes the bank).
            def rhs_rows(i, h0):
                # image rows needed by kernel-row i for out rows h0..h0+3
                return h0 + i - 1

            for t in range(N_TILES):
                h0 = t * TILE_ROWS
                ps = psum.tile([128, TILE_ROWS, W], f32)
                mms = []  # (lhsT, out_slice, rhs_ap, late)
                for i in (1, 0, 2):
                    for j in range(KW):
                        r = rhs_rows(i, h0)
                        lhsT = wts[:, i * KW + j, :]
                        if r < 0:
                            # t=0, i=0: out row 0 reads image row H-1
                            # (loaded+built first, so no reordering needed)
                            mms.append(
                                (lhsT, ps[:, 0:1, :], img[:, H - 1 : H, j : j + W], 0)
                            )
                            mms.append(
                                (lhsT, ps[:, 1:TILE_ROWS, :],
                                 img[:, 0 : TILE_ROWS - 1, j : j + W], 0)
                            )
                        elif r + TILE_ROWS > H:
                            # t=31, i=2: out row 3 reads image row 0
                            mms.append(
                                (lhsT, ps[:, 0 : TILE_ROWS - 1, :],
                                 img[:, r : H, j : j + W], 0)
                            )
                            mms.append(
                                (lhsT, ps[:, TILE_ROWS - 1 : TILE_ROWS, :],
                                 img[:, 0:1, j : j + W], 0)
                            )
                        else:
                            mms.append(
                                (lhsT, ps[:, :, :],
                                 img[:, r : r + TILE_ROWS, j : j + W], 0)
                            )
                mms.sort(key=lambda m: m[3])
                for n, (lhsT, out_sl, rhs, _late) in enumerate(mms):
                    nc.tensor.matmul(
                        out_sl,
                        lhsT=lhsT,
                        rhs=rhs,
                        start=(n == 0),
                        stop=(n == len(mms) - 1),
                    )

                # bias merge into a 32-row staging tile (stores are then 8KB
                # contiguous per partition -> 64 descriptors per DMA).
                # Alternate ACT and DVE so neither engine serializes the PE.
                g = t % OGROUP
                if g == 0:
                    st = ostage.tile([128, OGROUP * TILE_ROWS, W], f32)
                row0 = g * TILE_ROWS
                # merges live on ACT only: DVE must stay free for the image
                # builds, or the scheduler interleaves merges ahead of them
                # and the matmul stream stalls waiting for image rows.
                nc.scalar.activation(
                    out=st[:, row0 : row0 + TILE_ROWS, :],
                    in_=ps,
                    func=mybir.ActivationFunctionType.Identity,
                    bias=bias_sb,
                )
                if t == N_TILES - 5:
                    # flush the first half of the last group early so the
                    # end-of-kernel store tail is only 16 rows
                    nc.sync.dma_start(
                        out=y_bo[:, 96:112, :], in_=st[:, 0:16, :]
                    )
                if g == OGROUP - 1:
                    g0 = (t - OGROUP + 1) * TILE_ROWS
                    if t == N_TILES - 1:
                        nc.sync.dma_start(
                            out=y_bo[:, 112:128, :], in_=st[:, 16:32, :]
                        )
                    else:
                        nc.sync.dma_start(
                            out=y_bo[:, g0 : g0 + OGROUP * TILE_ROWS, :],
                            in_=st,
                        )
    return nc


def _get_module():
    if "nc" not in _CACHE:
        nc = _build_module()
        # CoreSim can't run modules with post-inserted instructions, so the
        # wait split is applied only on the hardware path.
        _split_multi_waits(nc)
        _CACHE["nc"] = nc
    return _CACHE["nc"]


def _in_maps(input, weight, bias):
    maps = []
    for i in range(N_CORES):
        lo, hi = i * BPC, (i + 1) * BPC
        # prebuild block-diag weights: wbd[64b+c, s, 64b+o] = w[b, c*9+s, o]
        wbd = np.zeros((128, KH * KW, 128), np.float32)
        wloc = weight[lo:hi].reshape(BPC, C, KH * KW, O)
        for b in range(BPC):
            wbd[64 * b : 64 * b + 64, :, 64 * b : 64 * b + 64] = wloc[b]
        maps.append(
            {
                "input": np.ascontiguousarray(input[lo:hi]),
                "wbd": wbd,
                "bias": np.ascontiguousarray(bias[lo:hi]),
            }
        )
    return maps


def kernel(input, weight, bias):
    from concourse.bass_utils import run_bass_kernel_spmd

    nc = _get_module()
    res = run_bass_kernel_spmd(
        nc, _in_maps(input, weight, bias), core_ids=list(range(N_CORES))
    )
    return np.concatenate([res.results[i]["out"] for i in range(N_CORES)], axis=0)



# revision 6
# speedup vs baseline: 1.3479x; 1.3479x over previous
"""Trainium2 Bass kernel for per-sample dynamic 3x3 conv (periodic padding).

y[b,o,h,w] = sum_{c,i,j} x[b,c,(h+i-1)%H,(w+j-1)%W] * wgt[b, c*9+i*3+j, o] + bias[b,o]

Shapes: x [16,64,128,128] f32, wgt [16,576,64] f32, bias [16,64] f32.
Sharding: data-parallel over batch, 2 samples per core on 8 cores.

Compute scheme: 64x64 PE-array tiling. Each matmul is K=64 (C), M=64 (O),
so four quadrant matmuls run concurrently on the 128x128 array:
  quadrant (row s, col g): sample s's image stream (SBUF partitions 64s..)
  contracting tap-set g's weights into PSUM partitions 64g..64g+64.
Per spatial tile (4 output rows, N=512) and per sample, the 9 taps are
split into set A (5 taps -> PSUM partitions 0:64) and set B (4 taps ->
64:128) accumulating in that sample's PSUM bank; the two half-sums are
merged (+bias) by one DVE scalar_tensor_tensor per sample. This doubles
useful PE throughput vs block-diagonal K=128 packing (all 128x128 MACs
are useful instead of half).

Periodic padding is handled with no padded-image build: matmuls read the
raw image and the column wrap (j=0/2) splits each tap into a 1-wide and a
127-wide matmul piece; the row wrap splits the boundary spatial tiles
(row H-1 is loaded first so tile 0 can start immediately).

Data are cast to bf16 host-side (inputs/weights) and the output is stored
bf16 and upcast on the host: halves DMA traffic so the ~30us PE stream
stays the critical path (HBM is ~424 GB/s aggregate across 16 engines).
"""

import numpy as np

KH = KW = 3
B, C, O, H, W = 16, 64, 64, 128, 128
N_CORES = 8
BPC = B // N_CORES  # samples per core
TILE_ROWS = 4  # output rows per PSUM tile -> N = 4*128 = 512
N_TILES = H // TILE_ROWS
OGROUP = 8  # spatial tiles per output store group (32 rows)

# tap sets per PE column-group: A -> PSUM partitions 0:64, B -> 64:128.
# First tap of each set has i=1 (never row-split), so the start=True
# matmul layout is identical for every spatial tile.
A_TAPS = [(1, 1), (0, 1), (2, 1), (0, 0), (2, 2)]
B_TAPS = [(1, 0), (1, 2), (2, 0), (0, 2)]

_CACHE = {}


def _patch_tile_drain():
    """This container's walrus rejects Drain instructions carrying more than
    one sem wait (setupSyncWait: Too many sync wait commands). Re-emit the
    TileContext exit drain's waits as individual wait_ge instructions."""
    import concourse.tile as tile
    from concourse.vector_clock import ScopedClock

    if getattr(tile.TileContext, "_drain_patch_applied", False):
        return

    def _drain_and_barrier(self, tick_clock, wait_clock):
        nc = self.nc
        nop = nc.sync.nop(nofuse=True)
        wait_clock.add_sem_waits(nop.ins, ScopedClock({None: tick_clock.global_clock}))
        waits = list(nop.ins.sync_info.on_wait)
        nop.ins.sync_info.on_wait.clear()
        assert self.sems is not None
        by_name = {}
        for h in self.sems.allocated().values():
            by_name[getattr(h, "name", None)] = h
        for w in waits:
            h = by_name.get(w.ant_name)
            assert h is not None, f"no sem handle for {w.ant_name}"
            nc.sync.wait_ge(h, w.wait_value)
        nc.sync.drain()
        nc.all_engine_barrier()
        popped = nc._tile_sem_poison_stack.pop()
        assert popped is self._sem_poison
        nc.clear_and_free_semaphores(list(self.sems.allocated().values()))

    tile.TileContext._drain_and_barrier = _drain_and_barrier
    tile.TileContext._drain_patch_applied = True


def _split_multi_waits(nc, max_waits=1):
    """Same walrus limitation, general form: any instruction carrying more
    than one sem wait fails setupSyncWait. Hoist excess waits onto dedicated
    single-wait NOPs on the same engine, placed just before the instruction."""
    import concourse.mybir as mybir

    for f in nc.m.functions:
        for blk in f.blocks:
            out = []
            changed = False
            for inst in blk.instructions:
                si = getattr(inst, "sync_info", None)
                waits = list(si.on_wait) if si is not None else []
                if len(waits) > max_waits:
                    changed = True
                    for w in waits[:-max_waits]:
                        out.append(
                            mybir.InstNoOp(
                                name=nc.get_next_instruction_name(),
                                engine=inst.engine,
                                sync_info=mybir.SyncInfo(on_wait=[w], on_update=[]),
                                bass_nofuse=True,
                            )
                        )
                    si.on_wait.clear()
                    for w in waits[-max_waits:]:
                        si.on_wait.append(w)
                out.append(inst)
            if changed:
                blk.instructions = out


def _row_pieces(t, i):
    """(out_row0, out_row1, img_row0) pieces for kernel-row i of tile t."""
    r = t * TILE_ROWS + i - 1
    if r < 0:  # t=0, i=0: out row 0 reads image row H-1
        return [(0, 1, H - 1), (1, TILE_ROWS, 0)]
    if r + TILE_ROWS > H:  # t=31, i=2: out row 3 reads image row 0
        return [(0, TILE_ROWS - 1, r), (TILE_ROWS - 1, TILE_ROWS, 0)]
    return [(0, TILE_ROWS, r)]


def _col_pieces(j):
    """(out_col0, out_col1, img_col0) pieces for kernel-col j (wrap at W)."""
    if j == 0:
        return [(0, 1, W - 1), (1, W, 0)]
    if j == 2:
        return [(W - 1, W, 0), (0, W - 1, 1)]
    return [(0, W, 0)]


def _build_module():
    import concourse.bass as bass
    import concourse.mybir as mybir
    import concourse.tile as tile

    _patch_tile_drain()

    f32 = mybir.dt.float32
    bf16 = mybir.dt.bfloat16

    nc = bass.Bass()
    x_d = nc.dram_tensor("input", [BPC, C, H, W], bf16, kind="ExternalInput")
    # weights pre-transposed host-side: wts[64*b+c, tap, o]
    w_d = nc.dram_tensor("wts", [128, KH * KW, O], bf16, kind="ExternalInput")
    b_d = nc.dram_tensor("bias", [BPC, O], f32, kind="ExternalInput")
    y_d = nc.dram_tensor("out", [BPC, O, H, W], bf16, kind="ExternalOutput")

    with tile.TileContext(nc) as tc:
        from contextlib import ExitStack

        ctx = ExitStack()
        with ctx:
            persist = ctx.enter_context(tc.tile_pool(name="persist", bufs=1))
            psum = ctx.enter_context(tc.tile_pool(name="psum", bufs=4, space="PSUM"))
            ostage = ctx.enter_context(tc.tile_pool(name="ostage", bufs=2))

            # --- first image rows ahead of everything (row H-1 first: tile 0
            # reads it through the periodic wrap)
            raw = persist.tile([128, H, W], bf16)
            x_bc = x_d.rearrange("b c h w -> (b c) h w")
            for r0, nr in [(H - 1, 1), (0, 8)]:
                nc.sync.dma_start(
                    out=raw[:, r0 : r0 + nr, :], in_=x_bc[:, r0 : r0 + nr, :]
                )

            # --- weights (147KB) + bias ---
            wts = persist.tile([128, KH * KW, O], bf16)
            nc.sync.dma_start(out=wts, in_=w_d[:, :, :])
            bias_sb = persist.tile([128, 1], f32)
            nc.sync.dma_start(
                out=bias_sb,
                in_=b_d.rearrange("b o -> (b o)").rearrange("(p x) -> p x", x=1),
            )
            # ACT act-table preload (first use costs ~1.3us; hide in load phase)
            act_warm = persist.tile([128, 1], f32)
            nc.scalar.activation(
                out=act_warm, in_=bias_sb, func=mybir.ActivationFunctionType.Copy
            )

            # --- bulk image loads, small-to-large chunks so tile-0 compute
            # starts within a few us while the rest streams in behind it.
            # Full 128-partition DMAs keep all 16 engines at full rate.
            for r0, nr in [(8, 24), (32, 32), (64, H - 1 - 64)]:
                nc.sync.dma_start(
                    out=raw[:, r0 : r0 + nr, :], in_=x_bc[:, r0 : r0 + nr, :]
                )

            y_bo = y_d.rearrange("b o h w -> (b o) h w")

            # --- main loop: 32 spatial tiles of 4 output rows. Per tile,
            # 4 quadrant chains (sample s x tap-set g) interleaved tap-by-tap
            # so the PE runs all four 64x64 array tiles concurrently.
            for t in range(N_TILES):
                ps = [
                    psum.tile([128, TILE_ROWS, W], f32, name=f"ps_{s}")
                    for s in range(BPC)
                ]

                chains = {}  # (s, g) -> list of (lhsT, out, rhs)
                for s in range(BPC):
                    for g, taps in enumerate((A_TAPS, B_TAPS)):
                        mms = []
                        for i, j in taps:
                            lhsT = wts[64 * s : 64 * s + 64, i * KW + j, :]
                            for ro0, ro1, ir in _row_pieces(t, i):
                                nr = ro1 - ro0
                                for co0, co1, ic in _col_pieces(j):
                                    ncol = co1 - co0
                                    out = ps[s][
                                        64 * g : 64 * g + 64, ro0:ro1, co0:co1
                                    ]
                                    rhs = raw[
                                        64 * s : 64 * s + 64,
                                        ir : ir + nr,
                                        ic : ic + ncol,
                                    ]
                                    mms.append((lhsT, out, rhs))
                        chains[(s, g)] = mms

                # interleave: one tap per chain per pass (taps can be 1-4
                # pieces; pieces of a tap stay consecutive)
                emitted = {k: 0 for k in chains}
                npass = max(len(A_TAPS), len(B_TAPS))
                for p in range(npass):
                    for s in range(BPC):
                        for g, taps in enumerate((A_TAPS, B_TAPS)):
                            if p >= len(taps):
                                continue
                            i, j = taps[p]
                            npieces = len(_row_pieces(t, i)) * len(_col_pieces(j))
                            mms = chains[(s, g)]
                            k0 = emitted[(s, g)]
                            for k in range(k0, k0 + npieces):
                                lhsT, out, rhs = mms[k]
                                nc.tensor.matmul(
                                    out,
                                    lhsT=lhsT,
                                    rhs=rhs,
                                    start=(k == 0),
                                    stop=(k == len(mms) - 1),
                                )
                            emitted[(s, g)] = k0 + npieces

                # merge the two PSUM half-sums + bias into the bf16 staging
                # tile. The DVE can't read PSUM twice in one op, so: ACT
                # cross-copies the "away" half (partitions that don't match
                # the output channel slot) into the staging tile, then DVE
                # adds the "home" half + bias on aligned partitions.
                g8 = t % OGROUP
                if g8 == 0:
                    st = ostage.tile([128, OGROUP * TILE_ROWS, W], bf16)
                row0 = g8 * TILE_ROWS
                for s in range(BPC):
                    home = slice(64 * s, 64 * s + 64)
                    away = slice(64 - 64 * s, 128 - 64 * s)
                    dst = st[home, row0 : row0 + TILE_ROWS, :]
                    nc.scalar.activation(
                        out=dst,
                        in_=ps[s][away, :, :],
                        func=mybir.ActivationFunctionType.Copy,
                    )
                    nc.vector.scalar_tensor_tensor(
                        out=dst,
                        in0=ps[s][home, :, :],
                        scalar=bias_sb[home, :],
                        in1=dst,
                        op0=mybir.AluOpType.add,
                        op1=mybir.AluOpType.add,
                    )

                if t == N_TILES - 5:
                    # flush the first half of the last group early so the
                    # end-of-kernel store tail is only 16 rows
                    nc.sync.dma_start(out=y_bo[:, 96:112, :], in_=st[:, 0:16, :])
                if g8 == OGROUP - 1:
                    g0 = (t - OGROUP + 1) * TILE_ROWS
                    if t == N_TILES - 1:
                        nc.sync.dma_start(out=y_bo[:, 112:128, :], in_=st[:, 16:32, :])
                    else:
                        nc.sync.dma_start(
                            out=y_bo[:, g0 : g0 + OGROUP * TILE_ROWS, :], in_=st
                        )
    return nc


def _get_module():
    if "nc" not in _CACHE:
        nc = _build_module()
        _split_multi_waits(nc)
        _CACHE["nc"] = nc
    return _CACHE["nc"]


def _in_maps(input, weight, bias):
    import ml_dtypes

    bf16 = ml_dtypes.bfloat16
    maps = []
    for i in range(N_CORES):
        lo, hi = i * BPC, (i + 1) * BPC
        # wts[64b+c, tap, o] = w[b, c*9+tap, o]
        wloc = weight[lo:hi].reshape(BPC, C, KH * KW, O)
        maps.append(
            {
                "input": np.ascontiguousarray(input[lo:hi]).astype(bf16),
                "wts": np.ascontiguousarray(wloc.reshape(BPC * C, KH * KW, O)).astype(
                    bf16
                ),
                "bias": np.ascontiguousarray(bias[lo:hi]),
            }
        )
    return maps


def kernel(input, weight, bias):
    from concourse.bass_utils import run_bass_kernel_spmd

    nc = _get_module()
    res = run_bass_kernel_spmd(
        nc, _in_maps(input, weight, bias), core_ids=list(range(N_CORES))
    )
    return np.concatenate(
        [res.results[i]["out"] for i in range(N_CORES)], axis=0
    ).astype(np.float32)


# revision 7
# speedup vs baseline: 1.6023x; 1.1888x over previous
"""Trainium2 Bass kernel for per-sample dynamic 3x3 conv (periodic padding).

y[b,o,h,w] = sum_{c,i,j} x[b,c,(h+i-1)%H,(w+j-1)%W] * wgt[b, c*9+i*3+j, o] + bias[b,o]

Shapes: x [16,64,128,128] f32, wgt [16,576,64] f32, bias [16,64] f32.
Sharding: data-parallel over batch, 2 samples per core on 8 cores.

Compute scheme: 64x64 PE-array tiling. Every matmul is K=64 (C), M=64 (O),
so four quadrant matmuls run concurrently on the 128x128 array — full
useful-MAC utilization (vs 50% for block-diagonal K=128 packing). The
quadrant grid over a spatial tile of 8 output rows:
  array rows (rhs stream): sample s lives in SBUF partitions 64s:64s+64
  array cols (PSUM rows):  col-group g computes output rows 4g:4g+4
so each quadrant (s,g) accumulates the FULL 9-tap sum for its own quarter
of the output — no cross-quadrant reduction. PSUM layout per spatial tile
is one [128, 2, 4, W] tile (2 banks; sample = bank, quadrant = partition
half x bank), every quadrant owning a private (partition x bank) region so
all four accumulation chains start/stop independently.

The tap loop runs OUTERMOST over a group of 2 spatial tiles so each
quadrant's stationary weights are loaded once per 2 matmuls (amortizing
LDWEIGHTS); PSUM holds 2 groups in flight (8 banks) so groups pipeline
without stalls.

Periodic padding needs no padded-image build: matmuls read the raw image;
the column wrap (j=0/2) splits each tap into a 1-wide and a 127-wide
piece, the row wrap splits the two boundary tiles (image row H-1 loads
first so tile 0 starts immediately).

Evacuation per sample: the g=s quadrant is already on the output channel
partitions -> DVE tensor_scalar_add(+bias) straight into the bf16 staging
tile; the g=1-s quadrant needs a partition-crossing copy -> ACT
activation(Identity, +bias), which the DVE crossbar/ACT support for
64-partition ops. Inputs/weights are cast to bf16 host-side and the
output is stored bf16 and upcast on the host: DMA traffic halves so the
~31us PE stream is the critical path (HBM ~424 GB/s aggregate).
"""

import numpy as np

KH = KW = 3
B, C, O, H, W = 16, 64, 64, 128, 128
N_CORES = 8
BPC = B // N_CORES  # samples per core
TILE_ROWS = 8  # output rows per spatial tile (4 per quadrant col-group)
QROWS = TILE_ROWS // 2  # rows per quadrant -> N = 4*128 = 512
N_TILES = H // TILE_ROWS
G = 2  # spatial tiles per tap-outer group
OGROUP = 4  # spatial tiles per output store group (32 rows)

TAPS = [(1, 1), (0, 1), (2, 1), (1, 0), (1, 2), (0, 0), (0, 2), (2, 0), (2, 2)]

_CACHE = {}


def _patch_tile_drain():
    """This container's walrus rejects Drain instructions carrying more than
    one sem wait (setupSyncWait: Too many sync wait commands). Re-emit the
    TileContext exit drain's waits as individual wait_ge instructions."""
    import concourse.tile as tile
    from concourse.vector_clock import ScopedClock

    if getattr(tile.TileContext, "_drain_patch_applied", False):
        return

    def _drain_and_barrier(self, tick_clock, wait_clock):
        nc = self.nc
        nop = nc.sync.nop(nofuse=True)
        wait_clock.add_sem_waits(nop.ins, ScopedClock({None: tick_clock.global_clock}))
        waits = list(nop.ins.sync_info.on_wait)
        nop.ins.sync_info.on_wait.clear()
        assert self.sems is not None
        by_name = {}
        for h in self.sems.allocated().values():
            by_name[getattr(h, "name", None)] = h
        for w in waits:
            h = by_name.get(w.ant_name)
            assert h is not None, f"no sem handle for {w.ant_name}"
            nc.sync.wait_ge(h, w.wait_value)
        nc.sync.drain()
        nc.all_engine_barrier()
        popped = nc._tile_sem_poison_stack.pop()
        assert popped is self._sem_poison
        nc.clear_and_free_semaphores(list(self.sems.allocated().values()))

    tile.TileContext._drain_and_barrier = _drain_and_barrier
    tile.TileContext._drain_patch_applied = True


def _split_multi_waits(nc, max_waits=1):
    """Same walrus limitation, general form: any instruction carrying more
    than one sem wait fails setupSyncWait. Hoist excess waits onto dedicated
    single-wait NOPs on the same engine, placed just before the instruction."""
    import concourse.mybir as mybir

    for f in nc.m.functions:
        for blk in f.blocks:
            out = []
            changed = False
            for inst in blk.instructions:
                si = getattr(inst, "sync_info", None)
                waits = list(si.on_wait) if si is not None else []
                if len(waits) > max_waits:
                    changed = True
                    for w in waits[:-max_waits]:
                        out.append(
                            mybir.InstNoOp(
                                name=nc.get_next_instruction_name(),
                                engine=inst.engine,
                                sync_info=mybir.SyncInfo(on_wait=[w], on_update=[]),
                                bass_nofuse=True,
                            )
                        )
                    si.on_wait.clear()
                    for w in waits[-max_waits:]:
                        si.on_wait.append(w)
                out.append(inst)
            if changed:
                blk.instructions = out


def _row_pieces(r0):
    """(out_row0, out_row1, img_row0) pieces for a quadrant block whose
    kernel-shifted image rows start at r0 (may wrap at either end)."""
    if r0 < 0:  # t=0, g=0, i=0: out row 0 reads image row H-1
        return [(0, 1, H - 1), (1, QROWS, 0)]
    if r0 + QROWS > H:  # t=last, g=1, i=2: last out row reads image row 0
        return [(0, QROWS - 1, r0), (QROWS - 1, QROWS, 0)]
    return [(0, QROWS, r0)]


def _col_pieces(j):
    """(out_col0, out_col1, img_col0) pieces for kernel-col j (wrap at W)."""
    if j == 0:
        return [(0, 1, W - 1), (1, W, 0)]
    if j == 2:
        return [(W - 1, W, 0), (0, W - 1, 1)]
    return [(0, W, 0)]


def _build_module():
    import concourse.bass as bass
    import concourse.mybir as mybir
    import concourse.tile as tile

    _patch_tile_drain()

    f32 = mybir.dt.float32
    bf16 = mybir.dt.bfloat16

    nc = bass.Bass()
    x_d = nc.dram_tensor("input", [BPC, C, H, W], bf16, kind="ExternalInput")
    # weights pre-transposed host-side: wts[64*b+c, tap, o]
    w_d = nc.dram_tensor("wts", [128, KH * KW, O], bf16, kind="ExternalInput")
    b_d = nc.dram_tensor("bias", [BPC, O], f32, kind="ExternalInput")
    y_d = nc.dram_tensor("out", [BPC, O, H, W], bf16, kind="ExternalOutput")

    with tile.TileContext(nc) as tc:
        from contextlib import ExitStack

        ctx = ExitStack()
        with ctx:
            persist = ctx.enter_context(tc.tile_pool(name="persist", bufs=1))
            psum = ctx.enter_context(tc.tile_pool(name="psum", bufs=2, space="PSUM"))
            ostage = ctx.enter_context(tc.tile_pool(name="ostage", bufs=2))

            # --- first image rows ahead of everything (row H-1 first: tile 0
            # reads it through the periodic wrap; group 0 reads rows 0-16)
            raw = persist.tile([128, H, W], bf16)
            x_bc = x_d.rearrange("b c h w -> (b c) h w")
            for r0, nr in [(H - 1, 1), (0, 17)]:
                nc.sync.dma_start(
                    out=raw[:, r0 : r0 + nr, :], in_=x_bc[:, r0 : r0 + nr, :]
                )

            # --- weights (147KB) + bias ---
            wts = persist.tile([128, KH * KW, O], bf16)
            nc.sync.dma_start(out=wts, in_=w_d[:, :, :])
            bias_sb = persist.tile([128, 1], f32)
            nc.sync.dma_start(
                out=bias_sb,
                in_=b_d.rearrange("b o -> (b o)").rearrange("(p x) -> p x", x=1),
            )
            # ACT act-table preload (first use costs ~1.3us; hide in load phase)
            act_warm = persist.tile([128, 1], f32)
            nc.scalar.activation(
                out=act_warm,
                in_=bias_sb,
                func=mybir.ActivationFunctionType.Identity,
                bias=bias_sb,
            )

            # --- bulk image loads, small-to-large chunks so tile-0 compute
            # starts within a few us while the rest streams in behind it.
            # Full 128-partition DMAs keep all 16 engines at full rate.
            for r0, nr in [(17, 24), (41, 32), (73, H - 1 - 73)]:
                nc.sync.dma_start(
                    out=raw[:, r0 : r0 + nr, :], in_=x_bc[:, r0 : r0 + nr, :]
                )

            y_bo = y_d.rearrange("b o h w -> (b o) h w")

            # --- main loop: groups of G spatial tiles, tap loop outermost
            # within a group so LDWEIGHTS amortizes over G matmuls.
            for t0 in range(0, N_TILES, G):
                tiles = list(range(t0, min(t0 + G, N_TILES)))
                ps = {
                    t: psum.tile([128, BPC, QROWS, W], f32, name=f"ps{t - t0}")
                    for t in tiles
                }

                # chains[(t, s, g)] -> list of (lhsT, out_ap, rhs_ap), in
                # tap-pass order (pieces of one tap stay consecutive)
                chains = {}
                for t in tiles:
                    for s in range(BPC):
                        for g in range(2):
                            mms = []
                            for i, j in TAPS:
                                lhsT = wts[64 * s : 64 * s + 64, i * KW + j, :]
                                r0 = t * TILE_ROWS + QROWS * g + i - 1
                                for ro0, ro1, ir in _row_pieces(r0):
                                    nr = ro1 - ro0
                                    for co0, co1, ic in _col_pieces(j):
                                        ncol = co1 - co0
                                        out = ps[t][
                                            64 * g : 64 * g + 64, s, ro0:ro1, co0:co1
                                        ]
                                        rhs = raw[
                                            64 * s : 64 * s + 64,
                                            ir : ir + nr,
                                            ic : ic + ncol,
                                        ]
                                        mms.append((lhsT, out, rhs))
                            chains[(t, s, g)] = mms

                emitted = {k: 0 for k in chains}
                for p, (i, j) in enumerate(TAPS):
                    for t in tiles:
                        for s in range(BPC):
                            for g in range(2):
                                r0 = t * TILE_ROWS + QROWS * g + i - 1
                                npieces = len(_row_pieces(r0)) * len(_col_pieces(j))
                                mms = chains[(t, s, g)]
                                k0 = emitted[(t, s, g)]
                                for k in range(k0, k0 + npieces):
                                    lhsT, out, rhs = mms[k]
                                    nc.tensor.matmul(
                                        out,
                                        lhsT=lhsT,
                                        rhs=rhs,
                                        start=(k == 0),
                                        stop=(k == len(mms) - 1),
                                    )
                                emitted[(t, s, g)] = k0 + npieces

                # --- evacuate each finished tile: per sample, the g=s
                # quadrant is partition-aligned with the output slot (DVE
                # +bias), the other quadrant crosses partitions (ACT +bias).
                for t in tiles:
                    g4 = t % OGROUP
                    if g4 == 0:
                        st = ostage.tile([128, OGROUP * TILE_ROWS, W], bf16)
                    row0 = g4 * TILE_ROWS
                    for s in range(BPC):
                        home = slice(64 * s, 64 * s + 64)
                        away = slice(64 - 64 * s, 128 - 64 * s)
                        nc.vector.tensor_scalar_add(
                            st[home, row0 + QROWS * s : row0 + QROWS * s + QROWS, :],
                            ps[t][home, s, :, :],
                            bias_sb[home, :],
                        )
                        nc.scalar.activation(
                            out=st[
                                home,
                                row0 + QROWS * (1 - s) : row0 + QROWS * (2 - s),
                                :,
                            ],
                            in_=ps[t][away, s, :, :],
                            func=mybir.ActivationFunctionType.Identity,
                            bias=bias_sb[home, :],
                        )

                    if t == N_TILES - 3:
                        # flush the first half of the last group early so the
                        # end-of-kernel store tail is only 16 rows
                        nc.sync.dma_start(out=y_bo[:, 96:112, :], in_=st[:, 0:16, :])
                    if g4 == OGROUP - 1:
                        gr0 = (t - OGROUP + 1) * TILE_ROWS
                        if t == N_TILES - 1:
                            nc.sync.dma_start(
                                out=y_bo[:, 112:128, :], in_=st[:, 16:32, :]
                            )
                        else:
                            nc.sync.dma_start(
                                out=y_bo[:, gr0 : gr0 + OGROUP * TILE_ROWS, :], in_=st
                            )
    return nc


def _get_module():
    if "nc" not in _CACHE:
        nc = _build_module()
        _split_multi_waits(nc)
        _CACHE["nc"] = nc
    return _CACHE["nc"]


def _in_maps(input, weight, bias):
    import ml_dtypes

    bf16 = ml_dtypes.bfloat16
    maps = []
    for i in range(N_CORES):
        lo, hi = i * BPC, (i + 1) * BPC
        # wts[64b+c, tap, o] = w[b, c*9+tap, o]
        wloc = weight[lo:hi].reshape(BPC, C, KH * KW, O)
        maps.append(
            {
                "input": np.ascontiguousarray(input[lo:hi]).astype(bf16),
                "wts": np.ascontiguousarray(wloc.reshape(BPC * C, KH * KW, O)).astype(
                    bf16
                ),
                "bias": np.ascontiguousarray(bias[lo:hi]),
            }
        )
    return maps


def kernel(input, weight, bias):
    from concourse.bass_utils import run_bass_kernel_spmd

    nc = _get_module()
    res = run_bass_kernel_spmd(
        nc, _in_maps(input, weight, bias), core_ids=list(range(N_CORES))
    )
    return np.concatenate(
        [res.results[i]["out"] for i in range(N_CORES)], axis=0
    ).astype(np.float32)
